# revision 10
# baseline (speedup 1.0000x reference)
"""Fused OT-DTW l2 cost-matrix kernel for Trainium2 (8 NeuronCores, SPMD).

mat_cost[i,j] = sum_{t,p,d} pi[cl(i)][t,p] * (X[i,t,d] - Y[j,p,d])^2
             = C1[i] + C2[cl(i), j] - 2 * C3[i,j]

with C3[i,j] = sum_{p,d} XP[i,p,d] * Y[j,p,d],  XP[i] = X[i].T @ pi[cl(i)].

pi is a 0/1 DTW path matrix (~770 nonzeros out of 262144), so XP costs
~100 MFLOP via sparse matmul on the host (exact, fp32), leaving the device
the single dense contraction C3: [1024, 65536] @ [65536, 1024] = 137 GFLOP
in fp8 DoubleRow (157 TF/s/core peak => ~109us/core floor).

Sharding: contraction-parallel (K-split) over the p axis. Core b takes
p in [64b, 64b+64): per core only 8.4MB of XP + 8.4MB of Y stream in --
each input byte is read exactly once across the machine (vs replicating
all of Y to every core, which is DMA-bound at ~190us). Each core emits a
full [1024, 1024] bf16 partial of C3; the host sums the 8 partials in
fp32 and applies the rank-1 corrections C1/C2 (0.15% of FLOPs).

Device layout (host pre-permuted so every DMA is contiguous):
  xpt [m, d, p, i']  fp8: XP^T m-blocks, xpt[m,d,p,i'] = XP[128m+i', 64b+p, d]
  yt  [d, p, j]      fp8: Y^T slice,     yt[d,p,j] = Y[j, 64b+p, d]
Per core: a short scratch-matmul burst warms the PE clock-gate while the
Y slice (8 chunked DMAs, sync queue) and first xpt m-block (scalar-engine
queue) stream in. Then 8 m-blocks x 32 DoubleRow p-pairs x 2 N-halves of
accumulating matmuls; PSUM [128,512] pairs double-buffered across m so the
DVE/ACT bf16 copies and the output DMA hide under the next block's matmuls.
"""

import os
import sys
import types

import numpy as np
import ml_dtypes

NX, NY, T, TP, D, C = 1024, 1024, 512, 512, 128, 8
N_CORES = 8
PSL = TP // N_CORES         # 64 p's per core (K-split)
MB = NX // 128              # 8 row blocks of 128
YG = 4                      # p's per Y chunk DMA
BF16 = ml_dtypes.bfloat16
F8 = ml_dtypes.float8_e4m3fn


def _ensure_axon_hooks():
    """concourse.bass_utils imports antenv.axon_hooks when tracing under
    axon; some images lack that submodule. Provide it, and register the
    NTFF profile hook if the boot path didn't."""
    try:
        import antenv
    except ImportError:
        return
    try:
        from antenv import axon_hooks  # noqa: F401
    except ImportError:
        mod = types.ModuleType("antenv.axon_hooks")
        mod._hook = None

        def _set(h):
            mod._hook = h

        def _get():
            return mod._hook

        mod.set_axon_ntff_profile_hook = _set
        mod.get_axon_ntff_profile_hook = _get
        sys.modules["antenv.axon_hooks"] = mod
        antenv.axon_hooks = mod
    from antenv.axon_hooks import (
        get_axon_ntff_profile_hook,
        set_axon_ntff_profile_hook,
    )

    if get_axon_ntff_profile_hook() is None:
        try:
            from trn_agent_boot.trn_boot import _ntff_profile_via_ctypes

            hook = _ntff_profile_via_ctypes("/opt/axon/libaxon_pjrt.so")
            if hook is not None:
                set_axon_ntff_profile_hook(hook)
        except Exception:
            pass


_ensure_axon_hooks()

import concourse.bass as bass  # noqa: E402, F401
import concourse.tile as tile  # noqa: E402
from concourse import bacc, mybir  # noqa: E402
from concourse.bass_utils import run_bass_kernel_spmd  # noqa: E402

_PROGRAM_CACHE = {}
LAST_RUN = None  # BassKernelResults of the most recent kernel() call


def _build_program():
    if "nc" in _PROGRAM_CACHE:
        return _PROGRAM_CACHE["nc"]
    f8 = mybir.dt.float8e4
    f32 = mybir.dt.float32
    bf16 = mybir.dt.bfloat16
    DR = mybir.MatmulPerfMode.DoubleRow
    nc = bacc.Bacc("TRN2", target_bir_lowering=False, debug=False,
                   num_devices=N_CORES)
    xpt = nc.dram_tensor("xpt", [MB, D, PSL, 128], f8, kind="ExternalInput").ap()
    yt = nc.dram_tensor("yt", [D, PSL, NY], f8, kind="ExternalInput").ap()
    c3p = nc.dram_tensor("c3p", [NX - 128, NY], bf16,
                         kind="ExternalOutput").ap()
    c3tail = nc.dram_tensor("c3tail", [2, 128, NY // 2], bf16,
                            kind="ExternalOutput").ap()

    with tile.TileContext(nc) as tc:
        with tc.tile_pool(name="sb", bufs=1) as sb_pool:
            # PE warmup: scratch matmuls at t=0 (values never read), so the
            # HAM clock-gate ramp starts immediately and overlaps the input
            # DMAs instead of delaying the real matmuls.
            with (
                tc.tile_pool(name="warm", bufs=1) as warm_pool,
                tc.tile_pool(name="warmps", bufs=1, space="PSUM") as warmps_pool,
            ):
                wsrc = warm_pool.tile([128, 512], f8)
                wacc = warmps_pool.tile([128, 512], f32)
                nc.vector.memset(wsrc[:], 0.0)
                for w in range(13):
                    nc.tensor.matmul(wacc[:], wsrc[:, 0:128], wsrc[:],
                                     start=True, stop=True)

            # All input DMAs issued upfront, hand-balanced across the two
            # HWDGE queues (sync + scalar/ACT, 8.4MB each) in deadline
            # order. Aggregate per-core HBM bandwidth caps at ~420 GB/s, so
            # the 16.8MB of input needs ~40us -- phase 1 below keeps the PE
            # fed while the stream completes. Compute starts as soon as the
            # first chunk lands (tile tracks per-slice deps).
            ysb = sb_pool.tile([D, PSL, NY], f8, tag="ysb")
            xms = [sb_pool.tile([D, PSL, 128], f8, tag="xm", bufs=MB,
                                name=f"xm{m}") for m in range(MB)]
            NCH = PSL // YG
            XC = PSL // 4      # xm chunk: 16 p's (256KB)

            def xm_chunk(m, c, eng):
                eng.dma_start(xms[m][:, c * XC:(c + 1) * XC, :],
                              xpt[m, :, c * XC:(c + 1) * XC, :])

            def y_chunk(g, eng):
                eng.dma_start(ysb[:, g * YG:(g + 1) * YG, :],
                              yt[:, g * YG:(g + 1) * YG, :])

            # Deadline-ordered issue: phase-1 round g consumes Y chunk g and
            # (every 4th round) the next 16-p slab of xm0..xm2, so those are
            # interleaved first; whole xm3..7 ride behind.
            # Critical path first: m0k0 needs only xm0c0 + Y0, on
            # opposite queues so neither waits behind the other.
            xm_chunk(0, 0, nc.scalar)
            y_chunk(0, nc.sync)
            xm_chunk(1, 0, nc.scalar)
            xm_chunk(2, 0, nc.sync)
            for c in range(4):
                if c > 0:
                    xm_chunk(0, c, nc.scalar)
                    xm_chunk(2, c, nc.sync)
                    xm_chunk(1, c, nc.scalar)
                for g in range(max(1, 4 * c), 4 * c + 4):
                    eng = nc.sync if g % 2 == 0 else nc.scalar
                    y_chunk(g, eng)
            for m, eng in ((3, nc.scalar), (4, nc.sync), (5, nc.scalar),
                           (6, nc.sync), (7, nc.scalar)):
                eng.dma_start(xms[m][:], xpt[m])

            KPC = YG // 2      # DR pairs per Y chunk
            NPH1 = 3           # blocks interleaved in phase 1
            with tc.tile_pool(name="psB", bufs=4, space="PSUM") as ps_pool:

                def emit(k, ps0, ps1, xm, only=None):
                    st, sp = (k == 0), (k == PSL // 2 - 1)
                    lhsT = xm[:, 2 * k:2 * k + 2, :]
                    if only != 1:
                        nc.tensor.matmul(ps0[:], lhsT,
                                         ysb[:, 2 * k:2 * k + 2, 0:512],
                                         start=st, stop=sp, perf_mode=DR)
                    if only != 0:
                        nc.tensor.matmul(ps1[:], lhsT,
                                         ysb[:, 2 * k:2 * k + 2, 512:1024],
                                         start=st, stop=sp, perf_mode=DR)

                def flush(m, ps0, ps1):
                    ob = out_pool_tile()
                    nc.vector.tensor_copy(ob[:, 0:512], ps0[:])
                    nc.scalar.copy(ob[:, 512:1024], ps1[:])
                    nc.sync.dma_start(c3p[m * 128:(m + 1) * 128, :], ob[:])

                def out_pool_tile():
                    return sb_pool.tile([128, NY], bf16, tag="ob", bufs=2,
                                        name="ob")

                # Phase 1: m=0..2 interleaved, paced by Y-chunk arrival: 12
                # matmuls per chunk keeps the PE busy through the stream-in
                # window and the HAM clock ramp.
                pss = [(ps_pool.tile([128, 512], f32, tag="ps0", name=f"ps0_{m}"),
                        ps_pool.tile([128, 512], f32, tag="ps1", name=f"ps1_{m}"))
                       for m in range(NPH1)]
                for g in range(NCH):
                    for m in range(NPH1):
                        for kk in range(KPC):
                            emit(g * KPC + kk, *pss[m], xms[m])
                for m in range(NPH1):
                    flush(m, *pss[m])

                # Phase 2: m=3..6, Y fully resident, pure PE streaming.
                for m in range(NPH1, MB - 1):
                    ps0 = ps_pool.tile([128, 512], f32, tag="ps0", name="ps0")
                    ps1 = ps_pool.tile([128, 512], f32, tag="ps1", name="ps1")
                    for k in range(PSL // 2):
                        emit(k, ps0, ps1, xms[m])
                    flush(m, ps0, ps1)

                # Last block: the two N-halves run as sequential groups so
                # half 0 drains (copy + contiguous c3tail DMA) underneath
                # half 1's matmuls; only half 1's short drain trails the
                # final matmul.
                ps0 = ps_pool.tile([128, 512], f32, tag="ps0", name="ps0")
                ps1 = ps_pool.tile([128, 512], f32, tag="ps1", name="ps1")
                m = MB - 1
                obt = out_pool_tile()
                for h in range(2):
                    for k in range(PSL // 2):
                        emit(k, ps0, ps1, xms[m], only=h)
                    ph = ps0 if h == 0 else ps1
                    o0 = 512 * h
                    nc.vector.tensor_copy(obt[:, o0:o0 + 256], ph[:, 0:256])
                    nc.scalar.copy(obt[:, o0 + 256:o0 + 512], ph[:, 256:512])
                    nc.sync.dma_start(c3tail[h], obt[:, o0:o0 + 512])

    nc.compile()
    _PROGRAM_CACHE["nc"] = nc
    return nc


def _host_xpt(X, pi, classe):
    """XPT[d, p, i] = sum_t pi[classe[i], t, p] * X[i, t, d], exact fp32.

    pi columns are ~1.5-sparse (0/1 DTW path), so this is ~100 MFLOP of
    sparse matmul instead of 69 GFLOP dense."""
    from scipy import sparse

    XPT = np.empty((D, TP, NX), dtype=np.float32)
    for c in range(C):
        rows = np.nonzero(classe == c)[0]
        if rows.size == 0:
            continue
        P = sparse.csr_matrix(pi[c])                              # [T, TP]
        Xt = np.ascontiguousarray(
            X[rows].transpose(1, 0, 2)).reshape(T, rows.size * D)
        XPc = (P.T @ Xt).reshape(TP, rows.size, D)
        XPT[:, :, rows] = XPc.transpose(2, 0, 1)
    return XPT


def kernel(X, Y, pi, classe):
    global LAST_RUN
    assert X.shape == (NX, T, D) and Y.shape == (NY, TP, D)
    assert pi.shape == (C, T, TP) and classe.shape == (NX,)
    X = np.asarray(X, dtype=np.float32)
    Y = np.asarray(Y, dtype=np.float32)
    pi = np.asarray(pi, dtype=np.float32)
    classe = np.asarray(classe)

    nc = _build_program()

    # Host prep: sparse XP, fp8 casts, per-core K-split slices with
    # m-block-contiguous layout so every device DMA is contiguous.
    XPT8 = _host_xpt(X, pi, classe).astype(F8)                    # [D, TP, NX]
    yt8 = np.ascontiguousarray(Y.transpose(2, 1, 0).astype(F8))   # [D, TP, NY]
    in_maps = []
    for b in range(N_CORES):
        ps = slice(b * PSL, (b + 1) * PSL)
        xk = np.ascontiguousarray(
            XPT8[:, ps, :].reshape(D, PSL, MB, 128).transpose(2, 0, 1, 3))
        yk = np.ascontiguousarray(yt8[:, ps, :])
        in_maps.append({"xpt": xk, "yt": yk})

    trace = bool(os.environ.get("BASS_TRACE"))
    LAST_RUN = run_bass_kernel_spmd(nc, in_maps, list(range(N_CORES)),
                                    trace=trace)
    C3 = np.zeros((NX, NY), dtype=np.float32)
    for b in range(N_CORES):
        r = LAST_RUN.results[b]
        C3[:NX - 128] += r["c3p"].astype(np.float32)
        C3[NX - 128:, 0:NY // 2] += r["c3tail"][0].astype(np.float32)
        C3[NX - 128:, NY // 2:] += r["c3tail"][1].astype(np.float32)

    # Host epilogue: rank-1 corrections (0.15% of FLOPs).
    row_c = pi.sum(-1)                                 # [C, T]
    col_c = pi.sum(1)                                  # [C, TP]
    SX = np.einsum("itd,itd->it", X, X)                # [NX, T]
    SY = np.einsum("jpd,jpd->jp", Y, Y)                # [NY, TP]
    C1 = np.einsum("it,it->i", SX, row_c[classe])      # [NX]
    C2 = col_c @ SY.T                                  # [C, NY]
    return (C1[:, None] + C2[classe] - 2.0 * C3).astype(np.float32)


# revision 11
# speedup vs baseline: 1.1752x; 1.1752x over previous
"""Fused OT-DTW l2 cost-matrix kernel for Trainium2 (8 NeuronCores, SPMD).

mat_cost[i,j] = sum_{t,p,d} pi[cl(i)][t,p] * (X[i,t,d] - Y[j,p,d])^2
             = C1[i] + C2[cl(i), j] - 2 * C3[i,j]

with C3[i,j] = sum_{p,d} XP[i,p,d] * Y[j,p,d],  XP[i] = X[i].T @ pi[cl(i)].

pi is a 0/1 DTW path matrix (~770 nonzeros out of 262144), so XP costs
~100 MFLOP via sparse matmul on the host (exact, fp32), leaving the device
the single dense contraction C3: [1024, 65536] @ [65536, 1024] = 137 GFLOP
in fp8 DoubleRow (157 TF/s/core peak => ~109us/core floor).

Sharding: contraction-parallel (K-split) over the p axis. Core b takes
p in [64b, 64b+64): per core only 8.4MB of XP + 8.4MB of Y stream in --
each input byte is read exactly once across the machine (vs replicating
all of Y to every core, which is DMA-bound at ~190us). Each core emits a
full [1024, 1024] bf16 partial of C3; the host sums the 8 partials in
fp32 and applies the rank-1 corrections C1/C2 (0.15% of FLOPs).

Device layout (host pre-permuted so every DMA is contiguous):
  xpt [m, d, p, i']  fp8: XP^T m-blocks, xpt[m,d,p,i'] = XP[128m+i', 64b+p, d]
  yt  [d, p, j]      fp8: Y^T slice,     yt[d,p,j] = Y[j, 64b+p, d]
Per core: a scratch-matmul burst warms the PE clock-gate from ~6.6us (any
PE idle during the HAM ramp extends its half-clock window) while the
inputs stream in on both HWDGE queues (sync + scalar/ACT; aggregate caps
at ~420 GB/s) in strict deadline order with the first chunks sliced fine.
Phase 1 interleaves m-blocks 0..2 (12 matmuls per Y chunk) so the PE
never starves while the 16.8MB stream completes; blocks 3..6 then run as
pure PE streaming (64 DoubleRow matmuls each, 216ns cadence, LDWEIGHTS
shadowed); the last block computes its two N-halves sequentially so half
0 drains under half 1's matmuls and only ~1.5us of drain trails the final
matmul. PSUM [128,512] accumulator pairs rotate over all 8 banks so the
DVE/ACT bf16 copies and output DMAs hide under the next block's matmuls.
"""

import os
import sys
import types

import numpy as np
import ml_dtypes

NX, NY, T, TP, D, C = 1024, 1024, 512, 512, 128, 8
N_CORES = 8
PSL = TP // N_CORES         # 64 p's per core (K-split)
MB = NX // 128              # 8 row blocks of 128
YG = 4                      # p's per Y chunk DMA
BF16 = ml_dtypes.bfloat16
F8 = ml_dtypes.float8_e4m3fn


def _ensure_axon_hooks():
    """concourse.bass_utils imports antenv.axon_hooks when tracing under
    axon; some images lack that submodule. Provide it, and register the
    NTFF profile hook if the boot path didn't."""
    try:
        import antenv
    except ImportError:
        return
    try:
        from antenv import axon_hooks  # noqa: F401
    except ImportError:
        mod = types.ModuleType("antenv.axon_hooks")
        mod._hook = None

        def _set(h):
            mod._hook = h

        def _get():
            return mod._hook

        mod.set_axon_ntff_profile_hook = _set
        mod.get_axon_ntff_profile_hook = _get
        sys.modules["antenv.axon_hooks"] = mod
        antenv.axon_hooks = mod
    from antenv.axon_hooks import (
        get_axon_ntff_profile_hook,
        set_axon_ntff_profile_hook,
    )

    if get_axon_ntff_profile_hook() is None:
        try:
            from trn_agent_boot.trn_boot import _ntff_profile_via_ctypes

            hook = _ntff_profile_via_ctypes("/opt/axon/libaxon_pjrt.so")
            if hook is not None:
                set_axon_ntff_profile_hook(hook)
        except Exception:
            pass


_ensure_axon_hooks()

import concourse.bass as bass  # noqa: E402, F401
import concourse.tile as tile  # noqa: E402
from concourse import bacc, mybir  # noqa: E402
from concourse.bass_utils import run_bass_kernel_spmd  # noqa: E402

_PROGRAM_CACHE = {}
LAST_RUN = None  # BassKernelResults of the most recent kernel() call


def _build_program():
    if "nc" in _PROGRAM_CACHE:
        return _PROGRAM_CACHE["nc"]
    f8 = mybir.dt.float8e4
    f32 = mybir.dt.float32
    bf16 = mybir.dt.bfloat16
    DR = mybir.MatmulPerfMode.DoubleRow
    nc = bacc.Bacc("TRN2", target_bir_lowering=False, debug=False,
                   num_devices=N_CORES)
    xpt = nc.dram_tensor("xpt", [MB, D, PSL, 128], f8, kind="ExternalInput").ap()
    yt = nc.dram_tensor("yt", [D, PSL, NY], f8, kind="ExternalInput").ap()
    c3p = nc.dram_tensor("c3p", [NX - 128, NY], bf16,
                         kind="ExternalOutput").ap()
    c3tail = nc.dram_tensor("c3tail", [2, 128, NY // 2], bf16,
                            kind="ExternalOutput").ap()

    with tile.TileContext(nc) as tc:
        with tc.tile_pool(name="sb", bufs=1) as sb_pool:
            # PE warmup: scratch matmuls at t=0 (values never read), so the
            # HAM clock-gate ramp starts immediately and overlaps the input
            # DMAs instead of delaying the real matmuls.
            with (
                tc.tile_pool(name="warm", bufs=1) as warm_pool,
                tc.tile_pool(name="warmps", bufs=1, space="PSUM") as warmps_pool,
            ):
                wsrc = warm_pool.tile([128, 512], f8)
                wacc = warmps_pool.tile([128, 512], f32)
                nc.vector.memset(wsrc[:], 0.0)
                for w in range(13):
                    nc.tensor.matmul(wacc[:], wsrc[:, 0:128], wsrc[:],
                                     start=True, stop=True)

            # All input DMAs issued upfront, hand-balanced across the two
            # HWDGE queues (sync + scalar/ACT, 8.4MB each) in deadline
            # order. Aggregate per-core HBM bandwidth caps at ~420 GB/s, so
            # the 16.8MB of input needs ~40us -- phase 1 below keeps the PE
            # fed while the stream completes. Compute starts as soon as the
            # first chunk lands (tile tracks per-slice deps).
            ysb = sb_pool.tile([D, PSL, NY], f8, tag="ysb")
            xms = [sb_pool.tile([D, PSL, 128], f8, tag="xm", bufs=MB,
                                name=f"xm{m}") for m in range(MB)]
            NCH = PSL // YG
            XC = PSL // 4      # xm chunk: 16 p's (256KB)

            def xm_chunk(m, c, eng):
                eng.dma_start(xms[m][:, c * XC:(c + 1) * XC, :],
                              xpt[m, :, c * XC:(c + 1) * XC, :])

            def y_chunk(g, eng):
                eng.dma_start(ysb[:, g * YG:(g + 1) * YG, :],
                              yt[:, g * YG:(g + 1) * YG, :])

            # Deadline-ordered issue: phase-1 round g consumes Y chunk g and
            # (every 4th round) the next 16-p slab of xm0..xm2, so those are
            # interleaved first; whole xm3..7 ride behind.
            # Critical path first: m0k0 needs only xm0c0 + Y0, on
            # opposite queues so neither waits behind the other.
            xm_chunk(0, 0, nc.scalar)
            y_chunk(0, nc.sync)
            xm_chunk(1, 0, nc.scalar)
            xm_chunk(2, 0, nc.sync)
            for c in range(4):
                if c > 0:
                    xm_chunk(0, c, nc.scalar)
                    xm_chunk(2, c, nc.sync)
                    xm_chunk(1, c, nc.scalar)
                for g in range(max(1, 4 * c), 4 * c + 4):
                    eng = nc.sync if g % 2 == 0 else nc.scalar
                    y_chunk(g, eng)
            for m, eng in ((3, nc.scalar), (4, nc.sync), (5, nc.scalar),
                           (6, nc.sync), (7, nc.scalar)):
                eng.dma_start(xms[m][:], xpt[m])

            KPC = YG // 2      # DR pairs per Y chunk
            NPH1 = 3           # blocks interleaved in phase 1
            with tc.tile_pool(name="psB", bufs=4, space="PSUM") as ps_pool:

                def emit(k, ps0, ps1, xm, only=None):
                    st, sp = (k == 0), (k == PSL // 2 - 1)
                    lhsT = xm[:, 2 * k:2 * k + 2, :]
                    if only != 1:
                        nc.tensor.matmul(ps0[:], lhsT,
                                         ysb[:, 2 * k:2 * k + 2, 0:512],
                                         start=st, stop=sp, perf_mode=DR)
                    if only != 0:
                        nc.tensor.matmul(ps1[:], lhsT,
                                         ysb[:, 2 * k:2 * k + 2, 512:1024],
                                         start=st, stop=sp, perf_mode=DR)

                def flush(m, ps0, ps1):
                    ob = out_pool_tile()
                    nc.vector.tensor_copy(ob[:, 0:512], ps0[:])
                    nc.scalar.copy(ob[:, 512:1024], ps1[:])
                    nc.sync.dma_start(c3p[m * 128:(m + 1) * 128, :], ob[:])

                def out_pool_tile():
                    return sb_pool.tile([128, NY], bf16, tag="ob", bufs=2,
                                        name="ob")

                # Phase 1: m=0..2 interleaved, paced by Y-chunk arrival: 12
                # matmuls per chunk keeps the PE busy through the stream-in
                # window and the HAM clock ramp.
                pss = [(ps_pool.tile([128, 512], f32, tag="ps0", name=f"ps0_{m}"),
                        ps_pool.tile([128, 512], f32, tag="ps1", name=f"ps1_{m}"))
                       for m in range(NPH1)]
                for g in range(NCH):
                    for m in range(NPH1):
                        for kk in range(KPC):
                            emit(g * KPC + kk, *pss[m], xms[m])
                for m in range(NPH1):
                    flush(m, *pss[m])

                # Phase 2: m=3..6, Y fully resident, pure PE streaming.
                for m in range(NPH1, MB - 1):
                    ps0 = ps_pool.tile([128, 512], f32, tag="ps0", name="ps0")
                    ps1 = ps_pool.tile([128, 512], f32, tag="ps1", name="ps1")
                    for k in range(PSL // 2):
                        emit(k, ps0, ps1, xms[m])
                    flush(m, ps0, ps1)

                # Last block: the two N-halves run as sequential groups so
                # half 0 drains (copy + contiguous c3tail DMA) underneath
                # half 1's matmuls; only half 1's short drain trails the
                # final matmul.
                ps0 = ps_pool.tile([128, 512], f32, tag="ps0", name="ps0")
                ps1 = ps_pool.tile([128, 512], f32, tag="ps1", name="ps1")
                m = MB - 1
                obt = out_pool_tile()
                for h in range(2):
                    for k in range(PSL // 2):
                        emit(k, ps0, ps1, xms[m], only=h)
                    ph = ps0 if h == 0 else ps1
                    o0 = 512 * h
                    nc.vector.tensor_copy(obt[:, o0:o0 + 256], ph[:, 0:256])
                    nc.scalar.copy(obt[:, o0 + 256:o0 + 512], ph[:, 256:512])
                    nc.sync.dma_start(c3tail[h], obt[:, o0:o0 + 512])

    nc.compile()
    _PROGRAM_CACHE["nc"] = nc
    return nc


def _host_xpt(X, pi, classe):
    """XPT[d, p, i] = sum_t pi[classe[i], t, p] * X[i, t, d], exact fp32.

    pi columns are ~1.5-sparse (0/1 DTW path), so this is ~100 MFLOP of
    sparse matmul instead of 69 GFLOP dense."""
    from scipy import sparse

    XPT = np.empty((D, TP, NX), dtype=np.float32)
    for c in range(C):
        rows = np.nonzero(classe == c)[0]
        if rows.size == 0:
            continue
        P = sparse.csr_matrix(pi[c])                              # [T, TP]
        Xt = np.ascontiguousarray(
            X[rows].transpose(1, 0, 2)).reshape(T, rows.size * D)
        XPc = (P.T @ Xt).reshape(TP, rows.size, D)
        XPT[:, :, rows] = XPc.transpose(2, 0, 1)
    return XPT


def kernel(X, Y, pi, classe):
    global LAST_RUN
    assert X.shape == (NX, T, D) and Y.shape == (NY, TP, D)
    assert pi.shape == (C, T, TP) and classe.shape == (NX,)
    X = np.asarray(X, dtype=np.float32)
    Y = np.asarray(Y, dtype=np.float32)
    pi = np.asarray(pi, dtype=np.float32)
    classe = np.asarray(classe)

    nc = _build_program()

    # Host prep: sparse XP, fp8 casts, per-core K-split slices with
    # m-block-contiguous layout so every device DMA is contiguous.
    XPT8 = _host_xpt(X, pi, classe).astype(F8)                    # [D, TP, NX]
    yt8 = np.ascontiguousarray(Y.transpose(2, 1, 0).astype(F8))   # [D, TP, NY]
    in_maps = []
    for b in range(N_CORES):
        ps = slice(b * PSL, (b + 1) * PSL)
        xk = np.ascontiguousarray(
            XPT8[:, ps, :].reshape(D, PSL, MB, 128).transpose(2, 0, 1, 3))
        yk = np.ascontiguousarray(yt8[:, ps, :])
        in_maps.append({"xpt": xk, "yt": yk})

    trace = bool(os.environ.get("BASS_TRACE"))
    LAST_RUN = run_bass_kernel_spmd(nc, in_maps, list(range(N_CORES)),
                                    trace=trace)
    C3 = np.zeros((NX, NY), dtype=np.float32)
    for b in range(N_CORES):
        r = LAST_RUN.results[b]
        C3[:NX - 128] += r["c3p"].astype(np.float32)
        C3[NX - 128:, 0:NY // 2] += r["c3tail"][0].astype(np.float32)
        C3[NX - 128:, NY // 2:] += r["c3tail"][1].astype(np.float32)

    # Host epilogue: rank-1 corrections (0.15% of FLOPs).
    row_c = pi.sum(-1)                                 # [C, T]
    col_c = pi.sum(1)                                  # [C, TP]
    SX = np.einsum("itd,itd->it", X, X)                # [NX, T]
    SY = np.einsum("jpd,jpd->jp", Y, Y)                # [NY, TP]
    C1 = np.einsum("it,it->i", SX, row_c[classe])      # [NX]
    C2 = col_c @ SY.T                                  # [C, NY]
    return (C1[:, None] + C2[classe] - 2.0 * C3).astype(np.float32)


# revision 13
# speedup vs baseline: 1.1887x; 1.0114x over previous
"""Fused OT-DTW l2 cost-matrix kernel for Trainium2 (8 NeuronCores, SPMD).

mat_cost[i,j] = sum_{t,p,d} pi[cl(i)][t,p] * (X[i,t,d] - Y[j,p,d])^2
             = C1[i] + C2[cl(i), j] - 2 * C3[i,j]

with C3[i,j] = sum_{p,d} XP[i,p,d] * Y[j,p,d],  XP[i] = X[i].T @ pi[cl(i)].

pi is a 0/1 DTW path matrix (~770 nonzeros out of 262144), so XP costs
~100 MFLOP via sparse matmul on the host (exact, fp32), leaving the device
the single dense contraction C3: [1024, 65536] @ [65536, 1024] = 137 GFLOP
in fp8 DoubleRow (157 TF/s/core peak => ~109us/core floor).

Sharding: contraction-parallel (K-split) over the p axis. Core b takes
p in [64b, 64b+64): per core only 8.4MB of XP + 8.4MB of Y stream in --
each input byte is read exactly once across the machine (vs replicating
all of Y to every core, which is DMA-bound at ~190us). Each core emits a
full [1024, 1024] bf16 partial of C3; the host sums the 8 partials in
fp32 and applies the rank-1 corrections C1/C2 (0.15% of FLOPs).

Device layout (host pre-permuted so every DMA is contiguous):
  xpt [m, d, p, i']  fp8: XP^T m-blocks, xpt[m,d,p,i'] = XP[128m+i', 64b+p, d]
  yt  [d, p, j]      fp8: Y^T slice,     yt[d,p,j] = Y[j, 64b+p, d]
Per core: a scratch-matmul burst warms the PE clock-gate from ~6.6us (any
PE idle during the HAM ramp extends its half-clock window) while the
inputs stream in on both HWDGE queues (sync + scalar/ACT; aggregate caps
at ~420 GB/s) in strict deadline order with the first chunks sliced fine.
Phase 1 interleaves m-blocks 0..2 (12 matmuls per Y chunk) so the PE
never starves while the 16.8MB stream completes; blocks 3..6 then run as
pure PE streaming (64 DoubleRow matmuls each, 216ns cadence, LDWEIGHTS
shadowed); the last block computes its two N-halves sequentially so half
0 drains under half 1's matmuls and only ~1.5us of drain trails the final
matmul. PSUM [128,512] accumulator pairs rotate over all 8 banks so the
DVE/ACT bf16 copies and output DMAs hide under the next block's matmuls.
"""

import os
import sys
import types

import numpy as np
import ml_dtypes

NX, NY, T, TP, D, C = 1024, 1024, 512, 512, 128, 8
N_CORES = 8
PSL = TP // N_CORES         # 64 p's per core (K-split)
MB = NX // 128              # 8 row blocks of 128
YG = 4                      # p's per Y chunk DMA
BF16 = ml_dtypes.bfloat16
F8 = ml_dtypes.float8_e4m3fn


def _ensure_axon_hooks():
    """concourse.bass_utils imports antenv.axon_hooks when tracing under
    axon; some images lack that submodule. Provide it, and register the
    NTFF profile hook if the boot path didn't."""
    try:
        import antenv
    except ImportError:
        return
    try:
        from antenv import axon_hooks  # noqa: F401
    except ImportError:
        mod = types.ModuleType("antenv.axon_hooks")
        mod._hook = None

        def _set(h):
            mod._hook = h

        def _get():
            return mod._hook

        mod.set_axon_ntff_profile_hook = _set
        mod.get_axon_ntff_profile_hook = _get
        sys.modules["antenv.axon_hooks"] = mod
        antenv.axon_hooks = mod
    from antenv.axon_hooks import (
        get_axon_ntff_profile_hook,
        set_axon_ntff_profile_hook,
    )

    if get_axon_ntff_profile_hook() is None:
        try:
            from trn_agent_boot.trn_boot import _ntff_profile_via_ctypes

            hook = _ntff_profile_via_ctypes("/opt/axon/libaxon_pjrt.so")
            if hook is not None:
                set_axon_ntff_profile_hook(hook)
        except Exception:
            pass


_ensure_axon_hooks()

import concourse.bass as bass  # noqa: E402, F401
import concourse.tile as tile  # noqa: E402
from concourse import bacc, mybir  # noqa: E402
from concourse.bass_utils import run_bass_kernel_spmd  # noqa: E402

_PROGRAM_CACHE = {}
LAST_RUN = None  # BassKernelResults of the most recent kernel() call


def _build_program():
    if "nc" in _PROGRAM_CACHE:
        return _PROGRAM_CACHE["nc"]
    f8 = mybir.dt.float8e4
    f32 = mybir.dt.float32
    bf16 = mybir.dt.bfloat16
    DR = mybir.MatmulPerfMode.DoubleRow
    nc = bacc.Bacc("TRN2", target_bir_lowering=False, debug=False,
                   num_devices=N_CORES)
    xpt = nc.dram_tensor("xpt", [MB, D, PSL, 128], f8, kind="ExternalInput").ap()
    yt = nc.dram_tensor("yt", [D, PSL, NY], f8, kind="ExternalInput").ap()
    c3p = nc.dram_tensor("c3p", [NX - 128, NY], bf16,
                         kind="ExternalOutput").ap()
    c3tail = nc.dram_tensor("c3tail", [2, 128, NY // 2], bf16,
                            kind="ExternalOutput").ap()

    with tile.TileContext(nc) as tc:
        with tc.tile_pool(name="sb", bufs=1) as sb_pool:
            # PE warmup: scratch matmuls at t=0 (values never read), so the
            # HAM clock-gate ramp starts immediately and overlaps the input
            # DMAs instead of delaying the real matmuls.
            with (
                tc.tile_pool(name="warm", bufs=1) as warm_pool,
                tc.tile_pool(name="warmps", bufs=1, space="PSUM") as warmps_pool,
            ):
                wsrc = warm_pool.tile([128, 512], f8)
                wacc = warmps_pool.tile([128, 512], f32)
                nc.vector.memset(wsrc[:], 0.0)
                for w in range(10):
                    nc.tensor.matmul(wacc[:], wsrc[:, 0:128], wsrc[:],
                                     start=True, stop=True)

            # All input DMAs issued upfront, hand-balanced across the two
            # HWDGE queues (sync + scalar/ACT, 8.4MB each) in deadline
            # order. Aggregate per-core HBM bandwidth caps at ~420 GB/s, so
            # the 16.8MB of input needs ~40us -- phase 1 below keeps the PE
            # fed while the stream completes. Compute starts as soon as the
            # first chunk lands (tile tracks per-slice deps).
            ysb = sb_pool.tile([D, PSL, NY], f8, tag="ysb")
            xms = [sb_pool.tile([D, PSL, 128], f8, tag="xm", bufs=MB,
                                name=f"xm{m}") for m in range(MB)]
            NCH = PSL // YG
            XC = PSL // 4      # xm chunk: 16 p's (256KB)

            def xm_chunk(m, c, eng):
                eng.dma_start(xms[m][:, c * XC:(c + 1) * XC, :],
                              xpt[m, :, c * XC:(c + 1) * XC, :])

            def y_chunk(g, eng):
                eng.dma_start(ysb[:, g * YG:(g + 1) * YG, :],
                              yt[:, g * YG:(g + 1) * YG, :])

            # Deadline-ordered issue: phase-1 round g consumes Y chunk g and
            # (every 4th round) the next 16-p slab of xm0..xm2, so those are
            # interleaved first; whole xm3..7 ride behind.
            # Critical path first: m0k0 needs only xm0c0 + Y0, on
            # opposite queues so neither waits behind the other.
            xm_chunk(0, 0, nc.scalar)
            y_chunk(0, nc.sync)
            xm_chunk(1, 0, nc.scalar)
            xm_chunk(2, 0, nc.sync)
            for c in range(4):
                if c > 0:
                    xm_chunk(0, c, nc.scalar)
                    xm_chunk(2, c, nc.sync)
                    xm_chunk(1, c, nc.scalar)
                for g in range(max(1, 4 * c), 4 * c + 4):
                    eng = nc.sync if g % 2 == 0 else nc.scalar
                    y_chunk(g, eng)
            for m, eng in ((3, nc.scalar), (4, nc.sync), (5, nc.scalar),
                           (6, nc.sync), (7, nc.scalar)):
                eng.dma_start(xms[m][:], xpt[m])

            KPC = YG // 2      # DR pairs per Y chunk
            NPH1 = 3           # blocks interleaved in phase 1
            with tc.tile_pool(name="psB", bufs=4, space="PSUM") as ps_pool:

                def emit(k, ps0, ps1, xm, only=None):
                    st, sp = (k == 0), (k == PSL // 2 - 1)
                    lhsT = xm[:, 2 * k:2 * k + 2, :]
                    if only != 1:
                        nc.tensor.matmul(ps0[:], lhsT,
                                         ysb[:, 2 * k:2 * k + 2, 0:512],
                                         start=st, stop=sp, perf_mode=DR)
                    if only != 0:
                        nc.tensor.matmul(ps1[:], lhsT,
                                         ysb[:, 2 * k:2 * k + 2, 512:1024],
                                         start=st, stop=sp, perf_mode=DR)

                def flush(m, ps0, ps1):
                    ob = out_pool_tile()
                    nc.vector.tensor_copy(ob[:, 0:512], ps0[:])
                    nc.scalar.copy(ob[:, 512:1024], ps1[:])
                    nc.sync.dma_start(c3p[m * 128:(m + 1) * 128, :], ob[:])

                def out_pool_tile():
                    return sb_pool.tile([128, NY], bf16, tag="ob", bufs=2,
                                        name="ob")

                # Phase 1: m=0..2 interleaved, paced by Y-chunk arrival: 12
                # matmuls per chunk keeps the PE busy through the stream-in
                # window and the HAM clock ramp.
                pss = [(ps_pool.tile([128, 512], f32, tag="ps0", name=f"ps0_{m}"),
                        ps_pool.tile([128, 512], f32, tag="ps1", name=f"ps1_{m}"))
                       for m in range(NPH1)]
                for g in range(NCH):
                    for m in range(NPH1):
                        for kk in range(KPC):
                            emit(g * KPC + kk, *pss[m], xms[m])
                for m in range(NPH1):
                    flush(m, *pss[m])

                # Phase 2: m=3..6, Y fully resident, pure PE streaming.
                for m in range(NPH1, MB - 1):
                    ps0 = ps_pool.tile([128, 512], f32, tag="ps0", name="ps0")
                    ps1 = ps_pool.tile([128, 512], f32, tag="ps1", name="ps1")
                    for k in range(PSL // 2):
                        emit(k, ps0, ps1, xms[m])
                    flush(m, ps0, ps1)

                # Last block: the two N-halves run as sequential groups so
                # half 0 drains (copy + contiguous c3tail DMA) underneath
                # half 1's matmuls; only half 1's short drain trails the
                # final matmul.
                ps0 = ps_pool.tile([128, 512], f32, tag="ps0", name="ps0")
                ps1 = ps_pool.tile([128, 512], f32, tag="ps1", name="ps1")
                m = MB - 1
                obt = out_pool_tile()
                for h in range(2):
                    for k in range(PSL // 2):
                        emit(k, ps0, ps1, xms[m], only=h)
                    ph = ps0 if h == 0 else ps1
                    o0 = 512 * h
                    nc.vector.tensor_copy(obt[:, o0:o0 + 256], ph[:, 0:256])
                    nc.scalar.copy(obt[:, o0 + 256:o0 + 512], ph[:, 256:512])
                    nc.sync.dma_start(c3tail[h], obt[:, o0:o0 + 512])

    nc.compile()
    _PROGRAM_CACHE["nc"] = nc
    return nc


def _host_xpt(X, pi, classe):
    """XPT[d, p, i] = sum_t pi[classe[i], t, p] * X[i, t, d], exact fp32.

    pi columns are ~1.5-sparse (0/1 DTW path), so this is ~100 MFLOP of
    sparse matmul instead of 69 GFLOP dense."""
    from scipy import sparse

    XPT = np.empty((D, TP, NX), dtype=np.float32)
    for c in range(C):
        rows = np.nonzero(classe == c)[0]
        if rows.size == 0:
            continue
        P = sparse.csr_matrix(pi[c])                              # [T, TP]
        Xt = np.ascontiguousarray(
            X[rows].transpose(1, 0, 2)).reshape(T, rows.size * D)
        XPc = (P.T @ Xt).reshape(TP, rows.size, D)
        XPT[:, :, rows] = XPc.transpose(2, 0, 1)
    return XPT


def kernel(X, Y, pi, classe):
    global LAST_RUN
    assert X.shape == (NX, T, D) and Y.shape == (NY, TP, D)
    assert pi.shape == (C, T, TP) and classe.shape == (NX,)
    X = np.asarray(X, dtype=np.float32)
    Y = np.asarray(Y, dtype=np.float32)
    pi = np.asarray(pi, dtype=np.float32)
    classe = np.asarray(classe)

    nc = _build_program()

    # Host prep: sparse XP, fp8 casts, per-core K-split slices with
    # m-block-contiguous layout so every device DMA is contiguous.
    XPT8 = _host_xpt(X, pi, classe).astype(F8)                    # [D, TP, NX]
    yt8 = np.ascontiguousarray(Y.transpose(2, 1, 0).astype(F8))   # [D, TP, NY]
    in_maps = []
    for b in range(N_CORES):
        ps = slice(b * PSL, (b + 1) * PSL)
        xk = np.ascontiguousarray(
            XPT8[:, ps, :].reshape(D, PSL, MB, 128).transpose(2, 0, 1, 3))
        yk = np.ascontiguousarray(yt8[:, ps, :])
        in_maps.append({"xpt": xk, "yt": yk})

    trace = bool(os.environ.get("BASS_TRACE"))
    LAST_RUN = run_bass_kernel_spmd(nc, in_maps, list(range(N_CORES)),
                                    trace=trace)
    C3 = np.zeros((NX, NY), dtype=np.float32)
    for b in range(N_CORES):
        r = LAST_RUN.results[b]
        C3[:NX - 128] += r["c3p"].astype(np.float32)
        C3[NX - 128:, 0:NY // 2] += r["c3tail"][0].astype(np.float32)
        C3[NX - 128:, NY // 2:] += r["c3tail"][1].astype(np.float32)

    # Host epilogue: rank-1 corrections (0.15% of FLOPs).
    row_c = pi.sum(-1)                                 # [C, T]
    col_c = pi.sum(1)                                  # [C, TP]
    SX = np.einsum("itd,itd->it", X, X)                # [NX, T]
    SY = np.einsum("jpd,jpd->jp", Y, Y)                # [NY, TP]
    C1 = np.einsum("it,it->i", SX, row_c[classe])      # [NX]
    C2 = col_c @ SY.T                                  # [C, NY]
    return (C1[:, None] + C2[classe] - 2.0 * C3).astype(np.float32)
